# revision 1
# baseline (speedup 1.0000x reference)
"""Trainium2 Bass kernel for CapsNet dynamic routing (ClassCapsules).

Reference computation (B=256, R=1152, C=10, O=16, I=8, 3 routing iters):
    u_hat[b,r,c,o] = sum_i W[r,c,o,i] * x[b,r,i]
    b_ij = 0
    for it in 3:
        c_ij = softmax(b_ij, axis=1)                      # over c
        s = sum_r c_ij[r,c] * u_hat[b,r,c,o] + bias       # [B,C,O]
        v = squash(s)
        if it < 2:
            b_ij += mean_b sum_o u_hat[b,r,c,o] v[b,c,o]  # [R,C]
    return v[..., None]

u_hat ([B,R,C,O] = 189MB fp32) is never materialized.  Both routing
contractions are re-associated through the factorization
    s[b,co]    = x~[b,(ri)] @ (c∘W~)[(ri),(co)]
    agree[r,c] = sum_{i,o} W~[(ri),(co)] * G[(ri),(co)],
                 G = (1/B) x~^T v
with x~ = x viewed as [B, R*I] and W~ = W viewed as [R*I, C*O].

Distribution: COLLECTIVE-FREE full replication.  Measured on this part,
the first CC-engine collective in a NEFF starts its mesh only after a
highly variable 55-134us init/rendezvous, and each AllReduce costs
~10-13us serialized with the routing chain; per-core HW exec time is
first-to-last instruction on that core, so a kernel with no cross-core
sync pays neither the init lottery nor launch skew.  Every core
redundantly computes the full-batch routing state in fp16 matmuls
(fp32 PSUM), and the final iteration narrows to the core's own
32-batch output shard.  The mean_b 1/B is folded into the `sel`
i-summation matrix (entries 1/B instead of 1).
"""

import os
import sys
import types

sys.path.insert(0, "/opt/trn_rl_repo")

# Shim antenv.axon_hooks (absent on this image) so BASS_TRACE=1 profiling
# works through run_bass_kernel_spmd's axon path.  Harmless when unused.
try:
    import antenv.axon_hooks  # noqa: F401
except ImportError:
    try:
        _hooks = types.ModuleType("antenv.axon_hooks")
        _hooks._hook = None
        _hooks.set_axon_ntff_profile_hook = lambda h: setattr(_hooks, "_hook", h)
        _hooks.get_axon_ntff_profile_hook = lambda: _hooks._hook
        sys.modules["antenv.axon_hooks"] = _hooks
        import antenv
        antenv.axon_hooks = _hooks
        from trn_agent_boot.trn_boot import _ntff_profile_via_ctypes
        _hooks.set_axon_ntff_profile_hook(
            _ntff_profile_via_ctypes("/opt/axon/libaxon_pjrt.so")
        )
    except Exception:
        pass

import numpy as np

import concourse.bacc as bacc
import concourse.bass as bass
import concourse.tile as tile
from concourse import mybir
import concourse.bass_utils as _bass_utils
from concourse.bass_utils import run_bass_kernel_spmd

if os.environ.get("BASS_TRACE"):
    _bass_utils.upload_artifacts = lambda tmpdir: ""  # no bucket access here

LAST_RESULT = None

F32 = mybir.dt.float32
F16 = mybir.dt.float16
ALU = mybir.AluOpType
ACT = mybir.ActivationFunctionType

B, R, C, O, I = 256, 1152, 10, 16, 8
CO = C * O                      # 160
N_CORES = 8
RI = R * I                      # 9216
NG = RI // 128                  # 72 groups of 128 (r,i) rows
GG = 9                          # dma/load granularity: 9 groups per chunk
NGG = NG // GG                  # 9 chunks
NB = B // 128                   # 2 batch partition chunks
B_SHARD = B // N_CORES          # 32 batches output per core
ITERS = 3
RPG = 128 // I                  # 16 r's per group
PB = 9                          # p9 reduce block (groups)


def _squash(nc, eps_sb, t, n_part, nb, pool, out_ap, name):
    """out = t * n2/((1+n2)*sqrt(n2+eps)); t: [n_part, nb, CO], reduce
    over o.  out_ap must be an [n_part, nb, CO]-shaped AP."""
    nc_ = nb * C
    tf = t.rearrange("p nb co -> p (nb co)")
    sq = pool.tile([n_part, nb * CO], F32, tag="sq", name=f"sq_{name}")
    nc.vector.tensor_mul(sq, tf, tf)
    n2 = pool.tile([n_part, nc_], F32, tag="n2", name=f"n2_{name}")
    nc.vector.reduce_sum(
        n2, sq.rearrange("p (nb c o) -> p nb c o", nb=nb, c=C),
        axis=mybir.AxisListType.X,
    )
    rt = pool.tile([n_part, nc_], F32, tag="rt", name=f"rt_{name}")
    nc.scalar.activation(rt, n2, ACT.Sqrt, bias=eps_sb[:n_part])
    n2p1 = pool.tile([n_part, nc_], F32, tag="n2p1", name=f"n2p1_{name}")
    nc.vector.tensor_scalar_add(n2p1, n2, 1.0)
    den = pool.tile([n_part, nc_], F32, tag="den", name=f"den_{name}")
    nc.vector.tensor_mul(den, n2p1, rt)
    rec = pool.tile([n_part, nc_], F32, tag="rec", name=f"rec_{name}")
    nc.vector.reciprocal(rec, den)
    fac = pool.tile([n_part, nc_], F32, tag="fac", name=f"fac_{name}")
    nc.vector.tensor_mul(fac, n2, rec)
    fac_b = fac.rearrange(
        "p (nb c one) -> p nb c one", nb=nb, c=C
    ).broadcast_to([n_part, nb, C, O])
    nc.vector.tensor_tensor(
        out=out_ap.rearrange("p nb (c o) -> p nb c o", c=C),
        in0=t.rearrange("p nb (c o) -> p nb c o", c=C),
        in1=fac_b,
        op=ALU.mult,
    )


def build():
    nc = bacc.Bacc("TRN2", target_bir_lowering=False, debug=False,
                   num_devices=N_CORES)

    # fp16 inputs, host pre-packed so every SBUF partition reads one
    # contiguous DRAM block.
    xt_d = nc.dram_tensor("xt", [128, NG, B], F16, kind="ExternalInput")
    xb_d = nc.dram_tensor("xb", [NB, 128, NG, 128], F16, kind="ExternalInput")
    xo_d = nc.dram_tensor("xo", [128, NG, B_SHARD], F16, kind="ExternalInput")
    wg_d = nc.dram_tensor("wg", [128, NG, CO], F16, kind="ExternalInput")
    bias_d = nc.dram_tensor("biasf", [CO], F32, kind="ExternalInput")
    sel_d = nc.dram_tensor("sel", [128, RPG], F32, kind="ExternalInput")
    selT_d = nc.dram_tensor("selT", [RPG, 128], F32, kind="ExternalInput")
    y_d = nc.dram_tensor("y", [B_SHARD, CO], F32, kind="ExternalOutput")

    with tile.TileContext(nc) as tc:
        with (
            tc.tile_pool(name="singles", bufs=1) as singles,
            tc.tile_pool(name="cw_pool", bufs=2) as cw_pool,
            tc.tile_pool(name="work", bufs=2) as work,
            tc.tile_pool(name="small", bufs=2) as small,
            tc.tile_pool(name="psum_s", bufs=1, space="PSUM") as psum_s,
            tc.tile_pool(name="psum_g", bufs=2, space="PSUM") as psum_g,
            tc.tile_pool(name="psum_misc", bufs=1, space="PSUM") as psum_misc,
        ):
            # ---- small constants first ----
            biasb = singles.tile([128, CO], F32)
            nc.sync.dma_start(
                out=biasb,
                in_=bass.AP(tensor=bias_d, offset=0, ap=[[0, 128], [1, CO]]),
            )
            sel_sb = singles.tile([128, RPG], F32)
            nc.sync.dma_start(out=sel_sb, in_=sel_d[:, :])
            selT_sb = singles.tile([RPG, 128], F32)
            nc.sync.dma_start(out=selT_sb, in_=selT_d[:, :])
            eps_sb = singles.tile([128, 1], F32)
            nc.vector.memset(eps_sb, 1e-8)

            # ---- bulk loads, chunked for load/compute pipelining ----
            WG = []                                    # 9 x [128, 8, CO]
            XT = []                                    # 9 x [128, 8, B]
            for gg in range(NGG):
                w_t = singles.tile([128, GG, CO], F16, tag=f"wg{gg}",
                                   name=f"wg_sb{gg}")
                nc.sync.dma_start(out=w_t,
                                  in_=wg_d[:, gg * GG:(gg + 1) * GG, :])
                WG.append(w_t)
                x_t = singles.tile([128, GG, B], F16, tag=f"xt{gg}",
                                   name=f"xt_sb{gg}")
                nc.sync.dma_start(out=x_t,
                                  in_=xt_d[:, gg * GG:(gg + 1) * GG, :])
                XT.append(x_t)
            XB = []                                    # [kb][gg] [128, 8, 128]
            for kb in range(NB):
                tiles = []
                for gg in range(NGG):
                    t = singles.tile([128, GG, 128], F16, tag=f"xb{kb}_{gg}",
                                     name=f"xb_sb{kb}_{gg}")
                    nc.sync.dma_start(
                        out=t, in_=xb_d[kb, :, gg * GG:(gg + 1) * GG, :]
                    )
                    tiles.append(t)
                XB.append(tiles)
            XO = singles.tile([128, NG, B_SHARD], F16)
            nc.sync.dma_start(out=XO, in_=xo_d[:, :, :])

            esr = None   # [16, 720 + 72]: exp(b_ij) ++ 1/sum_c exp(b_ij)

            for it in range(ITERS):
                # ---- CW = c∘W~ (it>0); it=0 uses uniform c=0.1 folded
                # into the post-matmul scale.
                if it == 0:
                    CW = None
                else:
                    # Broadcast esr across partitions via PE: cp[p, col] =
                    # esr[p//8, col].  cols 0..719 = exp(b)[g,c],
                    # 720..791 = 1/sum_c exp(b) [g].
                    cp_sb = small.tile([128, NG * C + NG], F32, tag="cpart",
                                       name=f"cp_sb_{it}")
                    half = 400
                    for h, (lo, hi) in enumerate(((0, half),
                                                  (half, NG * C + NG))):
                        cp_ps = psum_misc.tile([128, half], F32, tag="cp",
                                               name=f"cp_ps_{it}_{h}")
                        nc.tensor.matmul(cp_ps[:, :hi - lo], selT_sb,
                                         esr[:, lo:hi], start=True,
                                         stop=True)
                        nc.scalar.copy(cp_sb[:, lo:hi], cp_ps[:, :hi - lo])
                    # c_ij = e * recip, batched in one DVE op
                    ee = small.tile([128, NG, C], F32, tag="ee",
                                    name=f"ee_{it}")
                    rec_b = cp_sb[:, NG * C:NG * C + NG].rearrange(
                        "p (g one) -> p g one", one=1
                    ).broadcast_to([128, NG, C])
                    nc.vector.tensor_tensor(
                        out=ee,
                        in0=cp_sb[:, :NG * C].rearrange(
                            "p (g c) -> p g c", g=NG),
                        in1=rec_b, op=ALU.mult,
                    )
                    # CW = W * c, one op per 8-group chunk (fp16 out)
                    CW = cw_pool.tile([128, NG, CO], F16, tag="cw",
                                      name=f"cw_{it}")
                    for gg in range(NGG):
                        ee_b = ee[:, gg * GG:(gg + 1) * GG, :].rearrange(
                            "p g (c one) -> p g c one", one=1
                        ).broadcast_to([128, GG, C, O])
                        nc.vector.tensor_tensor(
                            out=CW[:, gg * GG:(gg + 1) * GG, :].rearrange(
                                "p g (c o) -> p g c o", c=C),
                            in0=WG[gg].rearrange("p g (c o) -> p g c o",
                                                 c=C),
                            in1=ee_b, op=ALU.mult,
                        )

                if it < ITERS - 1:
                    # ---- s (full batch): [256,160] = x~^T @ CW ----
                    s_ps = [psum_s.tile([128, CO], F32, tag=f"s{kb}",
                                        name=f"s_ps{kb}_{it}")
                            for kb in range(NB)]
                    v_sb = work.tile([128, NB, CO], F16, tag="vsb",
                                     name=f"v_sb_{it}")
                    for kb in range(NB):
                        for g in range(NG):
                            cw_g = (WG[g // GG][:, g % GG, :] if it == 0
                                    else CW[:, g, :])
                            nc.tensor.matmul(
                                s_ps[kb],
                                XT[g // GG][:, g % GG,
                                            kb * 128:(kb + 1) * 128],
                                cw_g,
                                start=(g == 0),
                                stop=(g == NG - 1),
                            )
                        t = work.tile([128, 1, CO], F32, tag="t",
                                      name=f"t_{it}_{kb}")
                        bias_b = biasb.rearrange(
                            "p (one co) -> p one co", one=1
                        )
                        nc.vector.scalar_tensor_tensor(
                            out=t,
                            in0=s_ps[kb].rearrange("p (one co) -> p one co",
                                                   one=1),
                            scalar=(0.1 if it == 0 else 1.0),
                            in1=bias_b, op0=ALU.mult, op1=ALU.add,
                        )
                        _squash(nc, eps_sb, t, 128, 1, work,
                                v_sb[:, kb:kb + 1, :], f"{it}_{kb}")

                    # ---- G = x~^T v ; agree = (1/B) sum_io W∘G ----
                    # (the 1/B lives in sel's entries)
                    Q_all = small.tile([128, NG * C], F32, tag="qall",
                                       name=f"qall_{it}")
                    for g in range(NG):
                        # 3 groups' G share one PSUM bank -> one batched
                        # DVE multiply per 3 groups instead of 3
                        if g % 3 == 0:
                            g_ps = psum_g.tile([128, 3, CO], F32, tag="gps",
                                               name=f"g_ps_{it}_{g // 3}")
                        for kb in range(NB):
                            nc.tensor.matmul(
                                g_ps[:, g % 3, :],
                                XB[kb][g // GG][:, g % GG, :],
                                v_sb[:, kb, :],
                                start=(kb == 0),
                                stop=(kb == NB - 1),
                            )
                        if g % PB == 0:
                            p9 = work.tile([128, PB, CO], F32, tag="p9",
                                           name=f"p9_{it}_{g // PB}")
                        if g % 3 == 2:
                            j0 = (g % PB) - 2
                            nc.vector.tensor_tensor(
                                out=p9[:, j0:j0 + 3, :], in0=g_ps,
                                in1=WG[g // GG][:, j0:j0 + 3, :],
                                op=ALU.mult,
                            )
                        if g % PB == PB - 1:
                            lo = g - (PB - 1)
                            nc.vector.reduce_sum(
                                Q_all[:, lo * C:(g + 1) * C],
                                p9.rearrange("p g (c o) -> p (g c) o", c=C),
                                axis=mybir.AxisListType.X,
                            )

                    # ---- agree (i-sum via sel matmul), esr update ----
                    esr_prev = esr
                    esr = small.tile([RPG, NG * C + NG], F32, tag="esr",
                                     name=f"esr_{it}")
                    half_a = 512
                    for h, (lo, hi) in enumerate(((0, half_a),
                                                  (half_a, NG * C))):
                        agree_ps = psum_misc.tile([RPG, half_a], F32,
                                                  tag="agree",
                                                  name=f"agree_{it}_{h}")
                        nc.tensor.matmul(agree_ps[:, :hi - lo], sel_sb,
                                         Q_all[:, lo:hi],
                                         start=True, stop=True)
                        if it == 0:
                            nc.scalar.activation(esr[:, lo:hi],
                                                 agree_ps[:, :hi - lo],
                                                 ACT.Exp)
                        else:
                            eexp = small.tile([RPG, half_a], F32,
                                              tag="eexp",
                                              name=f"eexp_{it}_{h}")
                            nc.scalar.activation(eexp[:, :hi - lo],
                                                 agree_ps[:, :hi - lo],
                                                 ACT.Exp)
                            nc.vector.tensor_mul(
                                esr[:, lo:hi], esr_prev[:, lo:hi],
                                eexp[:, :hi - lo]
                            )
                    den = small.tile([RPG, NG], F32, tag="sden",
                                     name=f"den_{it}")
                    nc.vector.reduce_sum(
                        den,
                        esr[:, :NG * C].rearrange("p (g c) -> p g c", g=NG),
                        axis=mybir.AxisListType.X,
                    )
                    nc.vector.reciprocal(esr[:, NG * C:], den)
                else:
                    # ---- final iter: s for own 32-batch shard only ----
                    s2_ps = psum_s.tile([128, CO], F32, tag="s0",
                                        name="s2_ps")
                    for g in range(NG):
                        nc.tensor.matmul(
                            s2_ps[:B_SHARD, :],
                            XO[:, g, :],
                            CW[:, g, :],
                            start=(g == 0),
                            stop=(g == NG - 1),
                        )
                    t2 = work.tile([B_SHARD, 1, CO], F32, tag="ft")
                    bias_b1 = biasb[:B_SHARD, :].rearrange(
                        "p (one co) -> p one co", one=1
                    )
                    nc.vector.scalar_tensor_tensor(
                        out=t2,
                        in0=s2_ps[:B_SHARD, :].rearrange(
                            "p (one co) -> p one co", one=1),
                        scalar=1.0,
                        in1=bias_b1, op0=ALU.mult, op1=ALU.add,
                    )
                    v2 = work.tile([B_SHARD, 1, CO], F32, tag="v2")
                    _squash(nc, eps_sb, t2, B_SHARD, 1, work, v2[:, :, :],
                            "fin")
                    nc.sync.dma_start(
                        out=y_d[:, :],
                        in_=v2.rearrange("p one co -> p (one co)")
                    )

    nc.compile()
    return nc


_NC = None


def kernel(x: np.ndarray, W: np.ndarray, bias: np.ndarray) -> np.ndarray:
    global _NC
    if _NC is None:
        _NC = build()

    x = np.ascontiguousarray(x, dtype=np.float32)
    W = np.ascontiguousarray(W, dtype=np.float32)
    bias = np.ascontiguousarray(bias, dtype=np.float32)

    xf = x.reshape(B, RI)
    # XT: [p, g, b] with ri = g*128 + p
    xt9 = np.ascontiguousarray(
        xf.T.reshape(NG, 128, B).transpose(1, 0, 2).astype(np.float16)
    )
    # XB: [kb, p_b, g, col] with b = kb*128 + p_b, ri = g*128 + col
    xb9 = np.ascontiguousarray(
        xf.reshape(NB, 128, NG, 128).astype(np.float16)
    )
    # W~: [(r i), (c o)] -> [p, g, co]
    wk = W.transpose(0, 3, 1, 2).reshape(RI, CO)
    wg9 = np.ascontiguousarray(
        wk.reshape(NG, 128, CO).transpose(1, 0, 2).astype(np.float16)
    )
    biasf = bias.reshape(CO)
    # sel holds the mean_b 1/B fold: agree = sel^T(1/B-scaled) @ Q
    sel = np.zeros((128, RPG), dtype=np.float32)
    sel[np.arange(128), np.arange(128) // I] = 1.0 / B
    selT = np.zeros((RPG, 128), dtype=np.float32)
    selT[np.arange(128) // I, np.arange(128)] = 1.0

    in_maps = []
    for k in range(N_CORES):
        xo = np.ascontiguousarray(
            xt9[:, :, k * B_SHARD:(k + 1) * B_SHARD]
        )
        in_maps.append({
            "xt": xt9,
            "xb": xb9,
            "xo": xo,
            "wg": wg9,
            "biasf": biasf,
            "sel": sel,
            "selT": selT,
        })

    global LAST_RESULT
    res = run_bass_kernel_spmd(
        _NC, in_maps, list(range(N_CORES)),
        trace=bool(os.environ.get("BASS_TRACE")),
    )
    LAST_RESULT = res
    v = np.concatenate([res.results[k]["y"] for k in range(N_CORES)], axis=0)
    return v.reshape(B, C, O)[..., None].astype(np.float32)



# revision 5
# speedup vs baseline: 1.2844x; 1.2844x over previous
"""Trainium2 Bass kernel for CapsNet dynamic routing (ClassCapsules).

Reference (B=256, R=1152, C=10, O=16, I=8, 3 routing iters):
    u_hat[b,r,c,o] = sum_i W[r,c,o,i] * x[b,r,i]
    b_ij = 0
    for it in 3:
        c_ij = softmax(b_ij, axis=1)                      # over c
        s = sum_r c_ij[r,c] * u_hat[b,r,c,o] + bias       # [B,C,O]
        v = squash(s)
        if it < 2:
            b_ij += mean_b sum_o u_hat[b,r,c,o] v[b,c,o]  # [R,C]
    return v[..., None]

u_hat (189MB) is never materialized; both contractions are re-associated as
    s[b,(co)]  = x~[b,(ri)] @ (c o W~)[(ri),(co)]
    agree[r,c] = sum_{i,o} W~ o G,  G = (1/B) x~^T v.

Distribution: collective-free full replication (CC init on this part costs
55-134us + ~10us per AllReduce, far more than recomputing).  Every core
computes full-batch routing state; the final iteration computes only the
core's own 32-batch output shard.

Implementation notes (HW-calibrated):
  * All big matmuls are MIXED fp8e4(stationary x) x fp16(moving W/CW/v):
    at full PE p-state the pair cadence is ~70ns (stream-bound, LDWEIGHTS
    fully hidden), vs 133ns for DoubleRow chains (LDW row-rate-bound).
    fp8 is ONLY on x (and only for the routing iterations); the final
    iteration's s uses fp16 x, so output error stays ~1e-3.
  * o-major column order (co) = (o,c): every large DVE op then has a
    packed fp16 last dim -> 2x DVE rate.  Broadcasts (c_ij over o, fac
    over o) sit on 0-stride OUTER dims which keep 2x eligibility.
  * ACT engine (otherwise idle) evacuates all PSUM (G chunks, cp chunks)
    with fused scale+cast; GPSIMD is ~13ns/elem on TRN2 and is not used.
  * The agree->softmax->CW->next-s chain is chunked by 9-group blocks so
    s(it+1) accumulation starts while agree(it) is still finishing.
"""

import os
import sys
import types

sys.path.insert(0, "/opt/trn_rl_repo")

# Shim antenv.axon_hooks (absent on this image) so BASS_TRACE=1 profiling
# works through run_bass_kernel_spmd's axon path.  Harmless when unused.
try:
    import antenv.axon_hooks  # noqa: F401
except ImportError:
    try:
        _hooks = types.ModuleType("antenv.axon_hooks")
        _hooks._hook = None
        _hooks.set_axon_ntff_profile_hook = lambda h: setattr(_hooks, "_hook", h)
        _hooks.get_axon_ntff_profile_hook = lambda: _hooks._hook
        sys.modules["antenv.axon_hooks"] = _hooks
        import antenv
        antenv.axon_hooks = _hooks
        from trn_agent_boot.trn_boot import _ntff_profile_via_ctypes
        _hooks.set_axon_ntff_profile_hook(
            _ntff_profile_via_ctypes("/opt/axon/libaxon_pjrt.so")
        )
    except Exception:
        pass

import numpy as np
import ml_dtypes

import concourse.bacc as bacc
import concourse.bass as bass
import concourse.tile as tile
from concourse import mybir
import concourse.bass_utils as _bass_utils
from concourse.bass_utils import run_bass_kernel_spmd

if os.environ.get("BASS_TRACE"):
    _bass_utils.upload_artifacts = lambda tmpdir: ""  # no bucket access here

LAST_RESULT = None

F32 = mybir.dt.float32
F16 = mybir.dt.float16
F8 = mybir.dt.float8e4
ALU = mybir.AluOpType
ACT = mybir.ActivationFunctionType

B, R, C, O, I = 256, 1152, 10, 16, 8
CO = C * O                      # 160
N_CORES = 8
RI = R * I                      # 9216
NG = RI // 128                  # 72 groups of 128 (r,i) rows
GG = 9                          # groups per chunk
NCH = NG // GG                  # 8 chunks
NB = B // 128                   # 2 batch partition blocks
B_SHARD = B // N_CORES          # 32
RPG = 128 // I                  # 16 r-slots per partition group
SX = 8.0                        # x fp8 pre-scale
SG = 64.0 / (SX * B)            # G psum -> fp16 scale (1/32)
ISUM = 1.0 / 64.0               # sel entries: undo the 64, fold mean_b
ITERS = 3


def _squash(nc, eps_sb, biasb, pool, s_ps_list, scalar, nparts, v_t, name,
            v_dtype=F16):
    """v = squash(s_psum*scalar + bias) batched over the kb blocks.

    s_ps_list: list of [nparts, CO] PSUM APs; v_t: [nparts, nb, CO] out tile
    (o-major co).  Returns nothing; writes v_t."""
    nb = len(s_ps_list)
    t = pool.tile([nparts, nb, CO], F16, tag="t", name=f"t_{name}")
    bias_b = biasb[:nparts, :].rearrange("p (one co) -> p one co", one=1)
    for kb in range(nb):
        nc.vector.scalar_tensor_tensor(
            out=t[:, kb:kb + 1, :],
            in0=s_ps_list[kb].rearrange("p (one co) -> p one co", one=1),
            scalar=scalar, in1=bias_b, op0=ALU.mult, op1=ALU.add,
        )
    sq = pool.tile([nparts, nb, CO], F16, tag="sq", name=f"sq_{name}")
    nc.vector.tensor_mul(sq, t, t)
    n2 = pool.tile([nparts, nb, C], F32, tag="n2", name=f"n2_{name}")
    nc.vector.reduce_sum(
        n2, sq.rearrange("p nb (o c) -> p nb c o", o=O),
        axis=mybir.AxisListType.X,
    )
    n2f = n2.rearrange("p nb c -> p (nb c)")
    rt = pool.tile([nparts, nb * C], F32, tag="rt", name=f"rt_{name}")
    nc.scalar.activation(rt, n2f, ACT.Sqrt, bias=eps_sb[:nparts])
    den = pool.tile([nparts, nb * C], F32, tag="den", name=f"den_{name}")
    nc.vector.scalar_tensor_tensor(
        out=den, in0=n2f, scalar=1.0, in1=rt, op0=ALU.add, op1=ALU.mult,
    )
    rec = pool.tile([nparts, nb * C], F32, tag="rec", name=f"rec_{name}")
    nc.vector.reciprocal(rec, den)
    fac = pool.tile([nparts, nb * C], F16, tag="fac", name=f"fac_{name}")
    nc.vector.tensor_mul(fac, n2f, rec)
    fac_b = fac.rearrange("p (nb c) -> p nb c", nb=nb).rearrange(
        "p nb (one c) -> p nb one c", one=1).broadcast_to([nparts, nb, O, C])
    nc.vector.tensor_tensor(
        out=v_t.rearrange("p nb (o c) -> p nb o c", o=O),
        in0=t.rearrange("p nb (o c) -> p nb o c", o=O),
        in1=fac_b, op=ALU.mult,
    )


def build():
    nc = bacc.Bacc("TRN2", target_bir_lowering=False, debug=False,
                   num_devices=N_CORES)

    xt8_d = nc.dram_tensor("xt8", [128, NG, B], F8, kind="ExternalInput")
    xb8_d = nc.dram_tensor("xb8", [128, NG, NB, 128], F8, kind="ExternalInput")
    xo16_d = nc.dram_tensor("xo16", [128, NG, B_SHARD], F16,
                            kind="ExternalInput")
    w16_d = nc.dram_tensor("w16", [128, NG, CO], F16, kind="ExternalInput")
    bias_d = nc.dram_tensor("biasf", [CO], F32, kind="ExternalInput")
    sel_d = nc.dram_tensor("sel", [128, RPG], F16, kind="ExternalInput")
    selT_d = nc.dram_tensor("selT", [RPG, 128], F16, kind="ExternalInput")
    y_d = nc.dram_tensor("y", [B_SHARD, CO], F32, kind="ExternalOutput")

    with tile.TileContext(nc) as tc:
        with (
            tc.tile_pool(name="singles", bufs=1) as singles,
            tc.tile_pool(name="work", bufs=2) as work,
            tc.tile_pool(name="small", bufs=2) as small,
            tc.tile_pool(name="psum_s", bufs=1, space="PSUM") as psum_s,
            tc.tile_pool(name="psum_g", bufs=2, space="PSUM") as psum_g,
            tc.tile_pool(name="psum_m", bufs=2, space="PSUM") as psum_m,
        ):
            # ---- ACT table preloads (overlap the DMA wait) ----
            warm = singles.tile([128, 2], F32)
            nc.vector.memset(warm, 1.0)
            warm2 = singles.tile([128, 2], F32)
            nc.scalar.activation(warm2, warm, ACT.Exp)
            nc.scalar.activation(warm2, warm, ACT.Sqrt)
            nc.scalar.copy(warm2, warm)

            # ---- small constants ----
            biasb = singles.tile([128, CO], F32)
            nc.sync.dma_start(
                out=biasb,
                in_=bass.AP(tensor=bias_d, offset=0, ap=[[0, 128], [1, CO]]),
            )
            sel_sb = singles.tile([128, RPG], F16)
            nc.sync.dma_start(out=sel_sb, in_=sel_d[:, :])
            selT_sb = singles.tile([RPG, 128], F16)
            nc.sync.dma_start(out=selT_sb, in_=selT_d[:, :])
            eps_sb = singles.tile([128, 1], F32)
            nc.vector.memset(eps_sb, 1e-8)

            # ---- bulk loads, chunked ----
            W16, XT8 = [], []
            for ch in range(NCH):
                w_t = singles.tile([128, GG, CO], F16, tag=f"w{ch}",
                                   name=f"w16_{ch}")
                nc.sync.dma_start(out=w_t,
                                  in_=w16_d[:, ch * GG:(ch + 1) * GG, :])
                W16.append(w_t)
                x_t = singles.tile([128, GG, B], F8, tag=f"xt{ch}",
                                   name=f"xt8_{ch}")
                nc.sync.dma_start(out=x_t,
                                  in_=xt8_d[:, ch * GG:(ch + 1) * GG, :])
                XT8.append(x_t)
            XB8 = []
            for ch in range(NCH):
                b_t = singles.tile([128, GG, NB, 128], F8, tag=f"xb{ch}",
                                   name=f"xb8_{ch}")
                nc.sync.dma_start(out=b_t,
                                  in_=xb8_d[:, ch * GG:(ch + 1) * GG, :, :])
                XB8.append(b_t)
            XO16 = singles.tile([128, NG, B_SHARD], F16)
            nc.sync.dma_start(out=XO16, in_=xo16_d[:, :, :])

            esr = None    # [16, 720+72] fp16: exp(b_ij) ++ 1/sum_c exp(b_ij)
            v16 = None

            for it in range(ITERS):
                # ================= s stage =================
                if it == 0:
                    ee16 = None
                else:
                    # cp = partition-broadcast of esr via PE; then
                    # ee16[p,(g,c)] = exp(b) * recip  (c_ij in fp16)
                    cp16 = small.tile([128, NG * C + NG], F16, tag="cp",
                                      name=f"cp_{it}")
                    half = 396
                    for h, (lo, hi) in enumerate(((0, half),
                                                  (half, NG * C + NG))):
                        cp_ps = psum_m.tile([128, 400], F32, tag="m",
                                            name=f"cp_ps_{it}_{h}")
                        nc.tensor.matmul(cp_ps[:, :hi - lo], selT_sb,
                                         esr[:, lo:hi], start=True, stop=True)
                        nc.scalar.copy(cp16[:, lo:hi], cp_ps[:, :hi - lo])
                    ee16 = small.tile([128, NG, C], F16, tag="ee",
                                      name=f"ee_{it}")
                    rec_b = cp16[:, NG * C:].rearrange(
                        "p (g one) -> p g one", one=1).broadcast_to(
                        [128, NG, C])
                    nc.vector.tensor_tensor(
                        out=ee16,
                        in0=cp16[:, :NG * C].rearrange("p (g c) -> p g c",
                                                       g=NG),
                        in1=rec_b, op=ALU.mult,
                    )

                final = it == ITERS - 1
                if not final:
                    s_ps = [psum_s.tile([128, CO], F32, tag=f"s{kb}",
                                        name=f"s_ps{kb}_{it}")
                            for kb in range(NB)]
                else:
                    s_ps = [psum_s.tile([128, CO], F32, tag="s0",
                                        name="s2_ps")]

                for ch in range(NCH):
                    if it == 0:
                        rhs_t = W16[ch]
                    else:
                        # CW = W o c_ij, all-fp16-packed 2x DVE
                        rhs_t = work.tile([128, GG, CO], F16, tag="cw",
                                          name=f"cw_{it}_{ch}")
                        ee_b = ee16[:, ch * GG:(ch + 1) * GG, :].rearrange(
                            "p g (one c) -> p g one c", one=1).broadcast_to(
                            [128, GG, O, C])
                        nc.vector.tensor_tensor(
                            out=rhs_t.rearrange("p g (o c) -> p g o c", o=O),
                            in0=W16[ch].rearrange("p g (o c) -> p g o c",
                                                  o=O),
                            in1=ee_b, op=ALU.mult,
                        )
                    for gg in range(GG):
                        g = ch * GG + gg
                        if not final:
                            for kb in range(NB):
                                nc.tensor.matmul(
                                    s_ps[kb],
                                    XT8[ch][:, gg, kb * 128:(kb + 1) * 128],
                                    rhs_t[:, gg, :],
                                    start=(g == 0), stop=(g == NG - 1),
                                )
                        else:
                            nc.tensor.matmul(
                                s_ps[0][:B_SHARD, :],
                                XO16[:, g, :],
                                rhs_t[:, gg, :],
                                start=(g == 0), stop=(g == NG - 1),
                            )

                # ================= squash =================
                if not final:
                    scal = (0.1 / SX) if it == 0 else (1.0 / SX)
                    v16 = work.tile([128, NB, CO], F16, tag="v",
                                    name=f"v_{it}")
                    _squash(nc, eps_sb, biasb, work, s_ps, scal, 128, v16,
                            f"{it}")
                else:
                    yv = work.tile([B_SHARD, 1, CO], F32, tag="yv")
                    _squash(nc, eps_sb, biasb, work,
                            [s_ps[0][:B_SHARD, :]], 1.0, B_SHARD, yv, "fin",
                            v_dtype=F32)
                    nc.sync.dma_start(
                        out=y_d[:, :],
                        in_=yv.rearrange("p one co -> p (one co)"))
                    break

                # ================= G / agree =================
                q16 = small.tile([128, NG, C], F16, tag="q",
                                 name=f"q_{it}")
                esr_prev = esr
                esr = small.tile([RPG, NG * C + NG], F16, tag="esr",
                                 name=f"esr_{it}")
                ha = NG * C // 2
                for ch in range(NCH):
                    g16 = work.tile([128, GG, CO], F16, tag="g16",
                                    name=f"g16_{it}_{ch}")
                    for t3 in range(3):
                        g_ps = psum_g.tile([128, 3, CO], F32, tag="g",
                                           name=f"g_ps_{it}_{ch}_{t3}")
                        for j in range(3):
                            g = ch * GG + t3 * 3 + j
                            for kb in range(NB):
                                nc.tensor.matmul(
                                    g_ps[:, j, :],
                                    XB8[ch][:, t3 * 3 + j, kb, :],
                                    v16[:, kb, :],
                                    start=(kb == 0), stop=(kb == NB - 1),
                                )
                        nc.scalar.activation(
                            g16[:, t3 * 3:t3 * 3 + 3, :],
                            g_ps.rearrange("p a co -> p (a co)"),
                            ACT.Copy, scale=SG)
                    # P = W o G (fp16 2x), o-pair presum, then reduce
                    p16 = work.tile([128, GG, CO], F16, tag="p16",
                                    name=f"p16_{it}_{ch}")
                    nc.vector.tensor_mul(p16, W16[ch], g16)
                    h8 = work.tile([128, GG, O // 2, C], F16, tag="h8",
                                   name=f"h8_{it}_{ch}")
                    pv = p16.rearrange("p g (o c) -> p g o c", o=O)
                    nc.vector.tensor_tensor(
                        out=h8, in0=pv[:, :, 0:O // 2, :],
                        in1=pv[:, :, O // 2:O, :], op=ALU.add)
                    with nc.allow_low_precision(reason="Q agreement fp16"):
                        nc.vector.reduce_sum(
                            q16[:, ch * GG:(ch + 1) * GG, :],
                            h8.rearrange("p g o c -> p g c o"),
                            axis=mybir.AxisListType.X,
                        )
                    if ch == NCH // 2 - 1 or ch == NCH - 1:
                        h = 0 if ch == NCH // 2 - 1 else 1
                        lo, hi = (0, ha) if h == 0 else (ha, NG * C)
                        agree_ps = psum_m.tile([128, 400], F32, tag="m",
                                               name=f"agree_{it}_{h}")
                        nc.tensor.matmul(
                            agree_ps[:RPG, :hi - lo], sel_sb,
                            q16.rearrange("p g c -> p (g c)")[:, lo:hi],
                            start=True, stop=True)
                        if it == 0:
                            nc.scalar.activation(esr[:, lo:hi],
                                                 agree_ps[:RPG, :hi - lo],
                                                 ACT.Exp)
                        else:
                            ex = small.tile([RPG, ha], F16, tag="ex",
                                            name=f"ex_{it}_{h}")
                            nc.scalar.activation(ex, agree_ps[:RPG, :hi - lo],
                                                 ACT.Exp)
                            nc.vector.tensor_mul(esr[:, lo:hi],
                                                 esr_prev[:, lo:hi], ex)
                den = small.tile([RPG, NG], F32, tag="sden",
                                 name=f"den_{it}")
                nc.vector.reduce_sum(
                    den, esr[:, :NG * C].rearrange("p (g c) -> p g c", g=NG),
                    axis=mybir.AxisListType.X,
                )
                with nc.allow_low_precision(reason="softmax recip fp16"):
                    nc.vector.reciprocal(esr[:, NG * C:], den)

    nc.compile()
    return nc


_NC = None


def kernel(x: np.ndarray, W: np.ndarray, bias: np.ndarray) -> np.ndarray:
    global _NC
    if _NC is None:
        _NC = build()

    x = np.ascontiguousarray(x, dtype=np.float32)
    W = np.ascontiguousarray(W, dtype=np.float32)
    bias = np.ascontiguousarray(bias, dtype=np.float32)

    xf = x.reshape(B, RI)
    x8 = (xf * SX).astype(ml_dtypes.float8_e4m3fn)
    # xt8[p, g, b] = x8[b, g*128+p]
    xt8 = np.ascontiguousarray(
        x8.T.reshape(NG, 128, B).transpose(1, 0, 2))
    # xb8[pb, g, kb, col] = x8[kb*128+pb, g*128+col]
    xb8 = np.ascontiguousarray(
        x8.reshape(NB, 128, NG, 128).transpose(1, 2, 0, 3))
    # w16[p, g, (o,c)] = W~[g*128+p, (c,o)] in o-major column order
    wk = W.transpose(0, 3, 1, 2).reshape(RI, C, O)     # [(ri), c, o]
    w16 = np.ascontiguousarray(
        wk.reshape(NG, 128, C, O).transpose(1, 0, 3, 2).reshape(
            128, NG, CO).astype(np.float16))
    biasf = np.ascontiguousarray(bias.T.reshape(CO))   # o-major
    sel = np.zeros((128, RPG), dtype=np.float16)
    sel[np.arange(128), np.arange(128) // I] = ISUM
    selT = np.zeros((RPG, 128), dtype=np.float16)
    selT[np.arange(128) // I, np.arange(128)] = 1.0
    xt16 = xf.T.reshape(NG, 128, B).transpose(1, 0, 2).astype(np.float16)

    in_maps = []
    for k in range(N_CORES):
        xo16 = np.ascontiguousarray(
            xt16[:, :, k * B_SHARD:(k + 1) * B_SHARD])
        in_maps.append({
            "xt8": xt8,
            "xb8": xb8,
            "xo16": xo16,
            "w16": w16,
            "biasf": biasf,
            "sel": sel,
            "selT": selT,
        })

    global LAST_RESULT
    res = run_bass_kernel_spmd(
        _NC, in_maps, list(range(N_CORES)),
        trace=bool(os.environ.get("BASS_TRACE")),
    )
    LAST_RESULT = res
    # y columns are o-major: y[b, o*10+c] -> v[b, c, o]
    ys = [res.results[k]["y"].reshape(B_SHARD, O, C).transpose(0, 2, 1)
          for k in range(N_CORES)]
    v = np.concatenate(ys, axis=0)
    return v.reshape(B, C, O)[..., None].astype(np.float32)
